# revision 31
# baseline (speedup 1.0000x reference)
"""Trainium2 Bass kernel for nn_MinimizeEnergy (bond/angle/dihedral energies).

Strategy: data-parallel over the term axis (8 cores, equal shards). Host
marshals the gather: per term it emits edge-difference vectors (p_i - p_j)
as scaled fp16 planes (planar SoA layout, one [P, cols] plane per vector
component), plus fp16 eq / tol^2 planes. The device kernel does all the
math: norms, half-angle arctan for bond angles, dihedral cos via the
X/Y trig-free formulation, energy terms, and per-partition accumulation.

Numerics: fp16 throughout the elementwise pipeline (DVE 2x mode), fp32
reductions. Vectors are pre-scaled (bonds/angles 1/16, dihedrals 1/32) so
all intermediates stay in fp16 range; the scales cancel in the angle/dih
ratios and are undone inside the bond sqrt's free scale slot.

ACT table sets are grouped into three phases (sqrt -> reciprocal -> trig)
to pay only three ACT_TABLE_LOADs.
"""
import sys
for _p in ('/opt/trn_rl_repo',):
    if _p not in sys.path:
        sys.path.insert(0, _p)

import numpy as np
from contextlib import ExitStack

import concourse.bass as bass
import concourse.tile as tile
from concourse import bacc, mybir

F32 = mybir.dt.float32
F16 = mybir.dt.float16
AF = mybir.ActivationFunctionType
ALU = mybir.AluOpType
AX = mybir.AxisListType
PI = float(np.pi)
P = 128
N_CORES = 8

N_ATOMS = 2_000_000
N_BONDS = 2_000_000
N_ANGLES = 4_000_000
N_DIH = 2_000_000

SB = 1.0 / 8.0    # bond vector prescale
SA = 1.0 / 8.0    # angle vector prescale
SD = 1.0 / 8.0    # dihedral vector prescale

PAD_TOL2 = 1.0e3  # tol^2 for padding terms -> relu(...) == 0


def _cols(n_per_core):
    """Columns per partition, padded so every plane is 4B-aligned (cols
    multiple of 4)."""
    c = -(-n_per_core // P)
    return -(-c // 4) * 4


def build_kernel(nb, na, nd, dbg=False):
    """nb/na/nd: per-core column counts (terms per partition).
    Angle/dihedral pipelines run in 2 column-chunks so the per-chunk
    dependency chains overlap across engines."""
    nc = bacc.Bacc("TRN2", target_bir_lowering=False, debug=False,
                   num_devices=N_CORES)
    b_v = nc.dram_tensor("b_v", [P, 3 * nb], F16, kind="ExternalInput").ap()
    b_eq = nc.dram_tensor("b_eq", [P, nb], F16, kind="ExternalInput").ap()
    b_t2 = nc.dram_tensor("b_t2", [P, nb], F16, kind="ExternalInput").ap()
    a_v = nc.dram_tensor("a_v", [P, 6 * na], F16, kind="ExternalInput").ap()
    a_eq = nc.dram_tensor("a_eq", [P, na], F16, kind="ExternalInput").ap()
    a_t2 = nc.dram_tensor("a_t2", [P, na], F16, kind="ExternalInput").ap()
    d_v = nc.dram_tensor("d_v", [P, 9 * nd], F16, kind="ExternalInput").ap()
    d_eq = nc.dram_tensor("d_eq", [P, nd], F16, kind="ExternalInput").ap()
    partials = nc.dram_tensor("partials", [P, 8], F32, kind="ExternalOutput").ap()

    V = nc.vector      # DVE
    A = nc.scalar      # ACT
    G = nc.gpsimd      # Pool

    NC_A = 2           # angle chunks
    NC_D = 2           # dih chunks
    ha = na // NC_A
    hd = nd // NC_D

    with tile.TileContext(nc) as tc, ExitStack() as ctx:
        pers = ctx.enter_context(tc.tile_pool(name="pers", bufs=1))
        ainp = ctx.enter_context(tc.tile_pool(name="ainp", bufs=1))
        dinp = ctx.enter_context(tc.tile_pool(name="dinp", bufs=1))

        acc = pers.tile([P, 8], F32)
        V.memset(acc[:], 0.0)
        halfpi = pers.tile([P, 1], F32)
        V.memset(halfpi[:], PI / 2)

        ta_v = ainp.tile([P, 6 * na], F16)
        ta_eq = ainp.tile([P, na], F16)
        ta_t2 = ainp.tile([P, na], F16)
        td_v = dinp.tile([P, 9 * nd], F16)
        td_eq = dinp.tile([P, nd], F16)

        # ================= BONDS (own pool scope, freed after) =============
        with tc.tile_pool(name="binp", bufs=1) as binp:
            tb_v = binp.tile([P, 3 * nb], F16)
            tb_eq = binp.tile([P, nb], F16)
            tb_t2 = binp.tile([P, nb], F16)
            G.dma_start(tb_v[:, 0:nb], b_v[:, 0:nb])
            G.dma_start(tb_v[:, nb:2 * nb], b_v[:, nb:2 * nb])
            G.dma_start(tb_v[:, 2 * nb:3 * nb], b_v[:, 2 * nb:3 * nb])
            G.dma_start(tb_eq[:], b_eq[:, :])
            G.dma_start(tb_t2[:], b_t2[:, :])
            G.dma_start(td_eq[:], d_eq[:, :])
            G.dma_start(ta_v[:, 0:3 * na], a_v[:, 0:3 * na])
            G.dma_start(ta_v[:, 3 * na:6 * na], a_v[:, 3 * na:6 * na])
            G.dma_start(ta_eq[:], a_eq[:, :])
            G.dma_start(ta_t2[:], a_t2[:, :])
            G.dma_start(td_v[:, 0:3 * nd], d_v[:, 0:3 * nd])
            G.dma_start(td_v[:, 3 * nd:6 * nd], d_v[:, 3 * nd:6 * nd])
            G.dma_start(td_v[:, 6 * nd:9 * nd], d_v[:, 6 * nd:9 * nd])

            bn2 = binp.tile([P, nb], F16, name="bn2")
            btmp = binp.tile([P, nb], F16, name="btmp")
            bd = binp.tile([P, nb], F16, name="bd")
            bx = tb_v[:, 0:nb]
            by = tb_v[:, nb:2 * nb]
            bz = tb_v[:, 2 * nb:3 * nb]
            V.tensor_mul(bn2[:], bx, bx)
            V.tensor_mul(btmp[:], by, by)
            V.tensor_add(bn2[:], bn2[:], btmp[:])
            V.tensor_mul(btmp[:], bz, bz)
            V.tensor_add(bn2[:], bn2[:], btmp[:])
            A.activation(bd[:], bn2[:], AF.Sqrt, scale=1.0 / (SB * SB))
            V.tensor_sub(bd[:], bd[:], tb_eq[:])      # diff
            V.tensor_mul(bd[:], bd[:], bd[:])         # diff^2
            V.tensor_sub(bd[:], bd[:], tb_t2[:])      # - tol^2
            V.tensor_scalar(bd[:], bd[:], 0.0, None, ALU.max, ALU.add,
                            accum_out=acc[:, 0:1])    # relu + sum

        awrk = ctx.enter_context(tc.tile_pool(name="awrk", bufs=1))
        dwrk = ctx.enter_context(tc.tile_pool(name="dwrk", bufs=1))
        aP0 = awrk.tile([P, na], F16, name="aP0")   # n0 / nn / aden / aa
        aP1 = awrk.tile([P, na], F16, name="aP1")   # n1 / sqnn / ratio
        aP2 = awrk.tile([P, na], F16, name="aP2")   # tmp / m_ / r2 / asq
        aP3 = awrk.tile([P, na], F16, name="aP3")   # tmp / p_
        aP4 = awrk.tile([P, na], F16, name="aP4")   # d01 / q_
        aF0 = awrk.tile([P, na], F32, name="aF0")   # add_ ; dih den
        aF1 = awrk.tile([P, na], F32, name="aF1")   # arcp ; dih 1/den
        dP = [dwrk.tile([P, nd], F16, name=f"dP{i}") for i in range(9)]

        # ---------------- angle stages (per column-chunk) ----------------
        def a_s(ci):
            return slice(ci * ha, (ci + 1) * ha)

        def apl(k, s):
            return ta_v[:, k * na + s.start:k * na + s.stop]

        def angle_front(ci):
            s = a_s(ci)
            p0, p1, p2, p3, p4 = (t[:, s] for t in (aP0, aP1, aP2, aP3, aP4))
            f0, f1 = aF0[:, s], aF1[:, s]
            A.activation(p0, apl(0, s), AF.Square)                  # sqrt-set
            A.activation(p2, apl(1, s), AF.Square)
            V.tensor_add(p0, p0, p2)
            A.activation(p2, apl(2, s), AF.Square)
            V.tensor_add(p0, p0, p2)
            A.activation(p1, apl(3, s), AF.Square)
            A.activation(p3, apl(4, s), AF.Square)
            V.tensor_add(p1, p1, p3)
            A.activation(p3, apl(5, s), AF.Square)
            V.tensor_add(p1, p1, p3)
            V.tensor_mul(p2, apl(0, s), apl(3, s))
            V.tensor_mul(p4, apl(1, s), apl(4, s))
            V.tensor_add(p4, p4, p2)
            V.tensor_mul(p2, apl(2, s), apl(5, s))
            V.tensor_add(p4, p4, p2)                 # d01
            V.tensor_mul(p0, p0, p1)                 # nn
            A.activation(p1, p0, AF.Sqrt)            # sqnn          sqrt-set
            V.tensor_sub(p2, p1, p4)                 # m_
            V.tensor_scalar(p2, p2, 0.0, None, ALU.max)
            A.activation(p3, p2, AF.Sqrt)            # p_            sqrt-set
            V.tensor_add(p0, p1, p4)                 # aden
            V.tensor_scalar(p0, p0, 0.0, None, ALU.max)
            A.activation(p4, p0, AF.Sqrt)            # q_            sqrt-set
            A.activation(p2, p1, AF.Sqrt, scale=2.0)  # r2           sqrt-set
            V.scalar_tensor_tensor(f0, p2, 1e-4, p4, ALU.add, ALU.add)
            V.reciprocal_approx_fast(f1, f0)
            V.tensor_mul(p1, p3, f1)                 # ratio (f16*f32->f16)
            V.tensor_scalar(p1, p1, 1.0, None, ALU.min)

        def angle_tail(ci):
            s = a_s(ci)
            p0, p1, p2 = aP0[:, s], aP1[:, s], aP2[:, s]
            A.activation(p0, p1, AF.Arctan)                         # trig-set
            V.scalar_tensor_tensor(p0, p0, 4.0, ta_eq[:, s],
                                   ALU.mult, ALU.subtract)
            A.activation(p2, p0, AF.Square)                         # trig-set
            V.tensor_sub(p2, p2, ta_t2[:, s])
            V.tensor_scalar(p2, p2, 0.0, None, ALU.max, ALU.add,
                            accum_out=acc[:, 1 + ci:2 + ci])

        # ---------------- dih stages (per column-chunk) ----------------
        def d_s(ci):
            return slice(ci * hd, (ci + 1) * hd)

        def dpl(k, s):
            return td_v[:, k * nd + s.start:k * nd + s.stop]

        def dih_main(ci):
            s = d_s(ci)
            b0x, b0y, b0z = dpl(0, s), dpl(1, s), dpl(2, s)
            ux, uy, uz = dpl(3, s), dpl(4, s), dpl(5, s)
            b2x, b2y, b2z = dpl(6, s), dpl(7, s), dpl(8, s)
            dL2, dm0, dm1 = dP[0][:, s], dP[1][:, s], dP[2][:, s]
            db0b2, db0u, db2u = dP[3][:, s], dP[4][:, s], dP[5][:, s]
            dY, gm0, dL = dP[6][:, s], dP[7][:, s], dP[8][:, s]
            fA, fB = aF0[:, s], aF1[:, s]
            # b0.u on Pool
            G.tensor_mul(gm0, b0x, ux)
            G.tensor_mul(db0u, b0y, uy)
            G.tensor_add(db0u, db0u, gm0)
            G.tensor_mul(gm0, b0z, uz)
            G.tensor_add(db0u, db0u, gm0)
            # L2 via ACT squares (staged through dm0)
            A.activation(dL2, ux, AF.Square)                        # sqrt-set
            A.activation(dm0, uy, AF.Square)
            V.tensor_add(dL2, dL2, dm0)
            A.activation(dm0, uz, AF.Square)
            V.tensor_add(dL2, dL2, dm0)
            # b0.b2 on DVE
            V.tensor_mul(dm0, b0x, b2x)
            V.tensor_mul(db0b2, b0y, b2y)
            V.tensor_add(db0b2, db0b2, dm0)
            V.tensor_mul(dm0, b0z, b2z)
            V.tensor_add(db0b2, db0b2, dm0)
            # b2.u on DVE
            V.tensor_mul(dm0, b2x, ux)
            V.tensor_mul(db2u, b2y, uy)
            V.tensor_add(db2u, db2u, dm0)
            V.tensor_mul(dm0, b2z, uz)
            V.tensor_add(db2u, db2u, dm0)
            # Y = (u x b0) . b2
            V.tensor_mul(dm0, uy, b0z)
            V.tensor_mul(dm1, uz, b0y)
            V.tensor_sub(dm0, dm0, dm1)
            V.tensor_mul(dY, dm0, b2x)
            V.tensor_mul(dm0, uz, b0x)
            V.tensor_mul(dm1, ux, b0z)
            V.tensor_sub(dm0, dm0, dm1)
            V.tensor_mul(dm0, dm0, b2y)
            V.tensor_add(dY, dY, dm0)
            V.tensor_mul(dm0, ux, b0y)
            V.tensor_mul(dm1, uy, b0x)
            V.tensor_sub(dm0, dm0, dm1)
            V.tensor_mul(dm0, dm0, b2z)
            V.tensor_add(dY, dY, dm0)
            # X = L2*b0b2 - (b0.u)(b2.u)
            V.tensor_mul(db0b2, dL2, db0b2)
            G.tensor_mul(db0u, db0u, db2u)
            V.tensor_sub(db0b2, db0b2, db0u)         # X
            # den = X^2 + (L*Y)^2, rt = 1/sqrt(den) -> fp16
            A.activation(dL, dL2, AF.Sqrt)                          # sqrt-set
            V.tensor_mul(dY, dL, dY)                 # LY
            A.activation(fA, db0b2, AF.Square)                      # sqrt-set
            A.activation(fB, dY, AF.Square)                         # sqrt-set
            V.scalar_tensor_tensor(fA, fA, 1e-9, fB, ALU.add, ALU.add)
            V.reciprocal_approx_fast(fB, fA)
            A.activation(dL2, fB, AF.Sqrt)           # rt16          sqrt-set

        def dih_trig(ci):
            s = d_s(ci)
            dm0, dm1, db2u = dP[1][:, s], dP[2][:, s], dP[5][:, s]
            A.activation(db2u, td_eq[:, s], AF.Sin)                 # trig-set
            A.activation(dm1, td_eq[:, s], AF.Abs)
            A.activation(dm0, dm1, AF.Sin, scale=-1.0, bias=halfpi[:])

        def dih_tail(ci):
            s = d_s(ci)
            dm0, db2u = dP[1][:, s], dP[5][:, s]
            dX, dY, rt16 = dP[3][:, s], dP[6][:, s], dP[0][:, s]
            V.tensor_mul(dX, dX, dm0)                # nx = X*ceq
            V.tensor_mul(dY, dY, db2u)               # ny = LY*seq
            V.tensor_add(dX, dX, dY)                 # num
            V.scalar_tensor_tensor(dX, dX, 1.0, rt16,
                                   ALU.mult, ALU.mult,
                                   accum_out=acc[:, 3 + ci:4 + ci])

        for ci in range(NC_A):
            angle_front(ci)
        for ci in range(NC_D):
            dih_main(ci)
        for ci in range(NC_D):
            dih_trig(ci)
        angle_tail(0)
        dih_tail(0)
        angle_tail(1)
        dih_tail(1)

        A.dma_start(partials[:], acc[:])
    nc.compile()
    return nc


def _run_spmd(nc, in_maps):
    import os
    if os.environ.get("EK_SIM") == "1":
        from concourse.bass_interp import CoreSim
        results = []
        for m in in_maps:
            sim = CoreSim(nc)
            for k, v in m.items():
                sim.tensor(k)[:] = v
            sim.simulate()
            results.append({"partials": np.array(sim.tensor("partials"))})
        return results
    from concourse.bass_utils import run_bass_kernel_spmd
    res = run_bass_kernel_spmd(nc, in_maps, list(range(len(in_maps))))
    return res.results


_BUILD_CACHE = {}


def _get_kernel(nb, na, nd):
    key = (nb, na, nd)
    if key not in _BUILD_CACHE:
        _BUILD_CACHE[key] = build_kernel(nb, na, nd)
    return _BUILD_CACHE[key]


def _shard_pad(arr, n_pad_per_core, fill=0.0):
    """[N,...] -> list of 8 per-core arrays padded to n_pad_per_core."""
    n = arr.shape[0]
    per = n // N_CORES
    out = []
    for c in range(N_CORES):
        a = arr[c * per:(c + 1) * per]
        npad = n_pad_per_core - per
        if npad:
            pad = np.full((npad,) + a.shape[1:], fill, dtype=a.dtype)
            a = np.concatenate([a, pad])
        out.append(a)
    return out


def _planes16(vecs, cols, ncomp):
    """[n_pad, ncomp] fp32 -> [P, ncomp*cols] fp16 planar."""
    v = vecs.reshape(P, cols, ncomp).transpose(0, 2, 1)  # [P, ncomp, cols]
    return np.ascontiguousarray(v.reshape(P, ncomp * cols).astype(np.float16))


def kernel(pos, bond_idcs, bond_eq_val, bond_tolerance,
           angle_idcs, angle_eq_val, angle_tolerance,
           dih_idcs, dih_eq_val):
    pos = np.asarray(pos, dtype=np.float32)
    bond_idcs = np.asarray(bond_idcs)
    angle_idcs = np.asarray(angle_idcs)
    dih_idcs = np.asarray(dih_idcs)

    nb = _cols(N_BONDS // N_CORES)
    na = _cols(N_ANGLES // N_CORES)
    nd = _cols(N_DIH // N_CORES)
    nbp, nap, ndp = nb * P, na * P, nd * P

    # ---- bonds: D = p0 - p1 (scaled) ----
    bD = (pos[bond_idcs[:, 0]] - pos[bond_idcs[:, 1]]) * SB
    b_eq = np.asarray(bond_eq_val, np.float32)
    b_t2 = np.asarray(bond_tolerance, np.float32) ** 2
    bDs = _shard_pad(bD, nbp)
    beqs = _shard_pad(b_eq, nbp)
    bt2s = _shard_pad(b_t2, nbp, fill=PAD_TOL2)

    # ---- angles: B0 = p0 - p1, B1 = p2 - p1 (scaled) ----
    aP1 = pos[angle_idcs[:, 1]]
    aB0 = (pos[angle_idcs[:, 0]] - aP1) * SA
    aB1 = (pos[angle_idcs[:, 2]] - aP1) * SA
    del aP1
    aV = np.concatenate([aB0, aB1], axis=1)  # [N,6]
    del aB0, aB1
    a_eq = np.asarray(angle_eq_val, np.float32)
    a_t2 = np.asarray(angle_tolerance, np.float32) ** 2
    aVs = _shard_pad(aV, nap)
    del aV
    aeqs = _shard_pad(a_eq, nap)
    at2s = _shard_pad(a_t2, nap, fill=PAD_TOL2)

    # ---- dihedrals: B0 = p0 - p1, U = p2 - p1, B2 = p3 - p2 (scaled) ----
    dP1 = pos[dih_idcs[:, 1]]
    dP2 = pos[dih_idcs[:, 2]]
    dB0 = (pos[dih_idcs[:, 0]] - dP1) * SD
    dU = (dP2 - dP1) * SD
    dB2 = (pos[dih_idcs[:, 3]] - dP2) * SD
    del dP1, dP2
    dV = np.concatenate([dB0, dU, dB2], axis=1)  # [N,9]
    del dB0, dU, dB2
    d_eq = np.asarray(dih_eq_val, np.float32)
    dVs = _shard_pad(dV, ndp)
    del dV
    deqs = _shard_pad(d_eq, ndp)

    nc = _get_kernel(nb, na, nd)

    in_maps = []
    for c in range(N_CORES):
        in_maps.append({
            "b_v": _planes16(bDs[c], nb, 3),
            "b_eq": beqs[c].reshape(P, nb).astype(np.float16),
            "b_t2": bt2s[c].reshape(P, nb).astype(np.float16),
            "a_v": _planes16(aVs[c], na, 6),
            "a_eq": aeqs[c].reshape(P, na).astype(np.float16),
            "a_t2": at2s[c].reshape(P, na).astype(np.float16),
            "d_v": _planes16(dVs[c], nd, 9),
            "d_eq": deqs[c].reshape(P, nd).astype(np.float16),
        })

    results = _run_spmd(nc, in_maps)

    bond_sum = 0.0
    angle_sum = 0.0
    cos_sum = 0.0
    for c in range(N_CORES):
        p = results[c]["partials"].astype(np.float64)
        bond_sum += p[:, 0].sum()
        angle_sum += p[:, 1].sum() + p[:, 2].sum()
        cos_sum += p[:, 3].sum() + p[:, 4].sum()

    # padding terms contribute exactly 0 to all three sums
    bond_energy = 1000.0 * bond_sum / N_BONDS
    angle_energy = 150.0 * angle_sum / N_ANGLES
    dih_energy = (2.0 * N_DIH - 2.0 * cos_sum) / N_DIH
    total = bond_energy + angle_energy + dih_energy
    return (np.float32(total), np.float32(bond_energy),
            np.float32(angle_energy), np.float32(dih_energy))


# revision 32
# speedup vs baseline: 1.0106x; 1.0106x over previous
"""Trainium2 Bass kernel for nn_MinimizeEnergy (bond/angle/dihedral energies).

Strategy: data-parallel over the term axis (8 cores, equal shards). Host
marshals the gather: per term it emits edge-difference vectors (p_i - p_j)
as scaled fp16 planes (planar SoA layout, one [P, cols] plane per vector
component), plus fp16 eq / tol^2 planes. The device kernel does all the
math: norms, half-angle arctan for bond angles, dihedral cos via the
X/Y trig-free formulation, energy terms, and per-partition accumulation.

Numerics: fp16 throughout the elementwise pipeline (DVE 2x mode), fp32
reductions. Vectors are pre-scaled (bonds/angles 1/16, dihedrals 1/32) so
all intermediates stay in fp16 range; the scales cancel in the angle/dih
ratios and are undone inside the bond sqrt's free scale slot.

ACT table sets are grouped into three phases (sqrt -> reciprocal -> trig)
to pay only three ACT_TABLE_LOADs.
"""
import sys
for _p in ('/opt/trn_rl_repo',):
    if _p not in sys.path:
        sys.path.insert(0, _p)

import numpy as np
from contextlib import ExitStack

import concourse.bass as bass
import concourse.tile as tile
from concourse import bacc, mybir

F32 = mybir.dt.float32
F16 = mybir.dt.float16
AF = mybir.ActivationFunctionType
ALU = mybir.AluOpType
AX = mybir.AxisListType
PI = float(np.pi)
P = 128
N_CORES = 8

N_ATOMS = 2_000_000
N_BONDS = 2_000_000
N_ANGLES = 4_000_000
N_DIH = 2_000_000

SB = 1.0 / 8.0    # bond vector prescale
SA = 1.0 / 8.0    # angle vector prescale
SD = 1.0 / 8.0    # dihedral vector prescale

PAD_TOL2 = 1.0e3  # tol^2 for padding terms -> relu(...) == 0


def _cols(n_per_core):
    """Columns per partition, padded so every plane is 4B-aligned (cols
    multiple of 4)."""
    c = -(-n_per_core // P)
    return -(-c // 4) * 4


def build_kernel(nb, na, nd, dbg=False):
    """nb/na/nd: per-core column counts (terms per partition).
    Angle/dihedral pipelines run in 2 column-chunks so the per-chunk
    dependency chains overlap across engines."""
    nc = bacc.Bacc("TRN2", target_bir_lowering=False, debug=False,
                   num_devices=N_CORES)
    b_v = nc.dram_tensor("b_v", [P, 3 * nb], F16, kind="ExternalInput").ap()
    b_eq = nc.dram_tensor("b_eq", [P, nb], F16, kind="ExternalInput").ap()
    b_t2 = nc.dram_tensor("b_t2", [P, nb], F16, kind="ExternalInput").ap()
    a_v = nc.dram_tensor("a_v", [P, 6 * na], F16, kind="ExternalInput").ap()
    a_eq = nc.dram_tensor("a_eq", [P, na], F16, kind="ExternalInput").ap()
    a_t2 = nc.dram_tensor("a_t2", [P, na], F16, kind="ExternalInput").ap()
    d_v = nc.dram_tensor("d_v", [P, 9 * nd], F16, kind="ExternalInput").ap()
    d_eq = nc.dram_tensor("d_eq", [P, nd], F16, kind="ExternalInput").ap()
    partials = nc.dram_tensor("partials", [P, 8], F32, kind="ExternalOutput").ap()

    V = nc.vector      # DVE
    A = nc.scalar      # ACT
    G = nc.gpsimd      # Pool

    NC_A = 2           # angle chunks
    NC_D = 2           # dih chunks
    ha = na // NC_A
    hd = nd // NC_D

    with tile.TileContext(nc) as tc, ExitStack() as ctx:
        pers = ctx.enter_context(tc.tile_pool(name="pers", bufs=1))
        ainp = ctx.enter_context(tc.tile_pool(name="ainp", bufs=1))
        dinp = ctx.enter_context(tc.tile_pool(name="dinp", bufs=1))

        acc = pers.tile([P, 8], F32)
        V.memset(acc[:], 0.0)
        halfpi = pers.tile([P, 1], F32)
        V.memset(halfpi[:], PI / 2)

        ta_v = ainp.tile([P, 6 * na], F16)
        ta_eq = ainp.tile([P, na], F16)
        ta_t2 = ainp.tile([P, na], F16)
        td_v = dinp.tile([P, 9 * nd], F16)
        td_eq = dinp.tile([P, nd], F16)

        # ================= BONDS (own pool scope, freed after) =============
        with tc.tile_pool(name="binp", bufs=1) as binp:
            tb_v = binp.tile([P, 3 * nb], F16)
            tb_eq = binp.tile([P, nb], F16)
            tb_t2 = binp.tile([P, nb], F16)
            G.dma_start(tb_v[:, 0:nb], b_v[:, 0:nb])
            G.dma_start(tb_v[:, nb:2 * nb], b_v[:, nb:2 * nb])
            G.dma_start(tb_v[:, 2 * nb:3 * nb], b_v[:, 2 * nb:3 * nb])
            G.dma_start(tb_eq[:], b_eq[:, :])
            G.dma_start(tb_t2[:], b_t2[:, :])
            G.dma_start(td_eq[:], d_eq[:, :])
            G.dma_start(ta_v[:, 0:3 * na], a_v[:, 0:3 * na])
            G.dma_start(ta_v[:, 3 * na:6 * na], a_v[:, 3 * na:6 * na])
            G.dma_start(ta_eq[:], a_eq[:, :])
            G.dma_start(ta_t2[:], a_t2[:, :])
            G.dma_start(td_v[:, 0:3 * nd], d_v[:, 0:3 * nd])
            G.dma_start(td_v[:, 3 * nd:6 * nd], d_v[:, 3 * nd:6 * nd])
            G.dma_start(td_v[:, 6 * nd:9 * nd], d_v[:, 6 * nd:9 * nd])

            bn2 = binp.tile([P, nb], F16, name="bn2")
            btmp = binp.tile([P, nb], F16, name="btmp")
            bd = binp.tile([P, nb], F16, name="bd")
            bx = tb_v[:, 0:nb]
            by = tb_v[:, nb:2 * nb]
            bz = tb_v[:, 2 * nb:3 * nb]
            V.tensor_mul(bn2[:], bx, bx)
            V.tensor_mul(btmp[:], by, by)
            V.tensor_add(bn2[:], bn2[:], btmp[:])
            V.tensor_mul(btmp[:], bz, bz)
            V.tensor_add(bn2[:], bn2[:], btmp[:])
            A.activation(bd[:], bn2[:], AF.Sqrt, scale=1.0 / (SB * SB))
            V.tensor_sub(bd[:], bd[:], tb_eq[:])      # diff
            V.tensor_mul(bd[:], bd[:], bd[:])         # diff^2
            V.tensor_sub(bd[:], bd[:], tb_t2[:])      # - tol^2
            V.tensor_scalar(bd[:], bd[:], 0.0, None, ALU.max, ALU.add,
                            accum_out=acc[:, 0:1])    # relu + sum

        awrk = ctx.enter_context(tc.tile_pool(name="awrk", bufs=1))
        dwrk = ctx.enter_context(tc.tile_pool(name="dwrk", bufs=1))
        aP0 = awrk.tile([P, na], F16, name="aP0")   # n0 / nn / aden / aa
        aP1 = awrk.tile([P, na], F16, name="aP1")   # n1 / sqnn / ratio
        aP2 = awrk.tile([P, na], F16, name="aP2")   # tmp / m_ / r2 / asq
        aP3 = awrk.tile([P, na], F16, name="aP3")   # tmp / p_
        aP4 = awrk.tile([P, na], F16, name="aP4")   # d01 / q_
        aF0 = awrk.tile([P, na], F32, name="aF0")   # add_ ; dih den
        aF1 = awrk.tile([P, na], F32, name="aF1")   # arcp ; dih 1/den
        dP = [dwrk.tile([P, nd], F16, name=f"dP{i}") for i in range(9)]

        # ---------------- angle stages (per column-chunk) ----------------
        def a_s(ci):
            return slice(ci * ha, (ci + 1) * ha)

        def apl(k, s):
            return ta_v[:, k * na + s.start:k * na + s.stop]

        def angle_front(ci):
            s = a_s(ci)
            p0, p1, p2, p3, p4 = (t[:, s] for t in (aP0, aP1, aP2, aP3, aP4))
            f0, f1 = aF0[:, s], aF1[:, s]
            A.activation(p0, apl(0, s), AF.Square)                  # sqrt-set
            A.activation(p2, apl(1, s), AF.Square)
            V.tensor_add(p0, p0, p2)
            A.activation(p2, apl(2, s), AF.Square)
            V.tensor_add(p0, p0, p2)
            A.activation(p1, apl(3, s), AF.Square)
            A.activation(p3, apl(4, s), AF.Square)
            V.tensor_add(p1, p1, p3)
            A.activation(p3, apl(5, s), AF.Square)
            V.tensor_add(p1, p1, p3)
            V.tensor_mul(p2, apl(0, s), apl(3, s))
            V.tensor_mul(p4, apl(1, s), apl(4, s))
            V.tensor_add(p4, p4, p2)
            V.tensor_mul(p2, apl(2, s), apl(5, s))
            V.tensor_add(p4, p4, p2)                 # d01
            V.tensor_mul(p0, p0, p1)                 # nn
            A.activation(p1, p0, AF.Sqrt)            # sqnn          sqrt-set
            V.tensor_sub(p2, p1, p4)                 # m_
            V.tensor_scalar(p2, p2, 0.0, None, ALU.max)
            A.activation(p3, p2, AF.Sqrt)            # p_            sqrt-set
            V.tensor_add(p0, p1, p4)                 # aden
            V.tensor_scalar(p0, p0, 0.0, None, ALU.max)
            A.activation(p4, p0, AF.Sqrt)            # q_            sqrt-set
            A.activation(p2, p1, AF.Sqrt, scale=2.0)  # r2           sqrt-set
            V.scalar_tensor_tensor(f0, p2, 1e-4, p4, ALU.add, ALU.add)
            V.reciprocal_approx_fast(f1, f0)
            V.tensor_mul(p1, p3, f1)                 # ratio (f16*f32->f16)
            V.tensor_scalar(p1, p1, 1.0, None, ALU.min)

        def angle_tail(ci):
            s = a_s(ci)
            p0, p1, p2 = aP0[:, s], aP1[:, s], aP2[:, s]
            A.activation(p0, p1, AF.Arctan)                         # trig-set
            V.scalar_tensor_tensor(p0, p0, 4.0, ta_eq[:, s],
                                   ALU.mult, ALU.subtract)
            A.activation(p2, p0, AF.Square)                         # trig-set
            V.tensor_sub(p2, p2, ta_t2[:, s])
            V.tensor_scalar(p2, p2, 0.0, None, ALU.max, ALU.add,
                            accum_out=acc[:, 1 + ci:2 + ci])

        # ---------------- dih stages (per column-chunk) ----------------
        def d_s(ci):
            return slice(ci * hd, (ci + 1) * hd)

        def dpl(k, s):
            return td_v[:, k * nd + s.start:k * nd + s.stop]

        def dih_main(ci):
            s = d_s(ci)
            b0x, b0y, b0z = dpl(0, s), dpl(1, s), dpl(2, s)
            ux, uy, uz = dpl(3, s), dpl(4, s), dpl(5, s)
            b2x, b2y, b2z = dpl(6, s), dpl(7, s), dpl(8, s)
            dL2, dm0, dm1 = dP[0][:, s], dP[1][:, s], dP[2][:, s]
            db0b2, db0u, db2u = dP[3][:, s], dP[4][:, s], dP[5][:, s]
            dY, gm0, dL = dP[6][:, s], dP[7][:, s], dP[8][:, s]
            fA, fB = aF0[:, s], aF1[:, s]
            # b0.u on Pool
            G.tensor_mul(gm0, b0x, ux)
            G.tensor_mul(db0u, b0y, uy)
            G.tensor_add(db0u, db0u, gm0)
            G.tensor_mul(gm0, b0z, uz)
            G.tensor_add(db0u, db0u, gm0)
            # L2 via ACT squares (staged through dm0)
            A.activation(dL2, ux, AF.Square)                        # sqrt-set
            A.activation(dm0, uy, AF.Square)
            V.tensor_add(dL2, dL2, dm0)
            A.activation(dm0, uz, AF.Square)
            V.tensor_add(dL2, dL2, dm0)
            # b0.b2 on DVE
            V.tensor_mul(dm0, b0x, b2x)
            V.tensor_mul(db0b2, b0y, b2y)
            V.tensor_add(db0b2, db0b2, dm0)
            V.tensor_mul(dm0, b0z, b2z)
            V.tensor_add(db0b2, db0b2, dm0)
            # b2.u on DVE
            V.tensor_mul(dm0, b2x, ux)
            V.tensor_mul(db2u, b2y, uy)
            V.tensor_add(db2u, db2u, dm0)
            V.tensor_mul(dm0, b2z, uz)
            V.tensor_add(db2u, db2u, dm0)
            # Y = (u x b0) . b2
            V.tensor_mul(dm0, uy, b0z)
            V.tensor_mul(dm1, uz, b0y)
            V.tensor_sub(dm0, dm0, dm1)
            V.tensor_mul(dY, dm0, b2x)
            V.tensor_mul(dm0, uz, b0x)
            V.tensor_mul(dm1, ux, b0z)
            V.tensor_sub(dm0, dm0, dm1)
            V.tensor_mul(dm0, dm0, b2y)
            V.tensor_add(dY, dY, dm0)
            V.tensor_mul(dm0, ux, b0y)
            V.tensor_mul(dm1, uy, b0x)
            V.tensor_sub(dm0, dm0, dm1)
            V.tensor_mul(dm0, dm0, b2z)
            V.tensor_add(dY, dY, dm0)
            # X = L2*b0b2 - (b0.u)(b2.u)
            V.tensor_mul(db0b2, dL2, db0b2)
            G.tensor_mul(db0u, db0u, db2u)
            V.tensor_sub(db0b2, db0b2, db0u)         # X
            # den = X^2 + (L*Y)^2, rt = 1/sqrt(den) -> fp16
            A.activation(dL, dL2, AF.Sqrt)                          # sqrt-set
            V.tensor_mul(dY, dL, dY)                 # LY
            A.activation(fA, db0b2, AF.Square)                      # sqrt-set
            A.activation(fB, dY, AF.Square)                         # sqrt-set
            V.scalar_tensor_tensor(fA, fA, 1e-9, fB, ALU.add, ALU.add)
            V.reciprocal_approx_fast(fB, fA)
            A.activation(dL2, fB, AF.Sqrt)           # rt16          sqrt-set

        def dih_trig(ci):
            s = d_s(ci)
            dm0, dm1, db2u = dP[1][:, s], dP[2][:, s], dP[5][:, s]
            A.activation(db2u, td_eq[:, s], AF.Sin)                 # trig-set
            A.activation(dm1, td_eq[:, s], AF.Abs)
            A.activation(dm0, dm1, AF.Sin, scale=-1.0, bias=halfpi[:])

        def dih_tail(ci):
            s = d_s(ci)
            dm0, db2u = dP[1][:, s], dP[5][:, s]
            dX, dY, rt16 = dP[3][:, s], dP[6][:, s], dP[0][:, s]
            V.tensor_mul(dX, dX, dm0)                # nx = X*ceq
            V.tensor_mul(dY, dY, db2u)               # ny = LY*seq
            V.tensor_add(dX, dX, dY)                 # num
            V.scalar_tensor_tensor(dX, dX, 1.0, rt16,
                                   ALU.mult, ALU.mult,
                                   accum_out=acc[:, 3 + ci:4 + ci])

        for ci in range(NC_A):
            angle_front(ci)
        for ci in range(NC_D):
            dih_main(ci)
        for ci in range(NC_D):
            dih_trig(ci)
        for ci in range(NC_A):
            angle_tail(ci)
        for ci in range(NC_D):
            dih_tail(ci)

        A.dma_start(partials[:], acc[:])
    nc.compile()
    return nc


def _run_spmd(nc, in_maps):
    import os
    if os.environ.get("EK_SIM") == "1":
        from concourse.bass_interp import CoreSim
        results = []
        for m in in_maps:
            sim = CoreSim(nc)
            for k, v in m.items():
                sim.tensor(k)[:] = v
            sim.simulate()
            results.append({"partials": np.array(sim.tensor("partials"))})
        return results
    from concourse.bass_utils import run_bass_kernel_spmd
    res = run_bass_kernel_spmd(nc, in_maps, list(range(len(in_maps))))
    return res.results


_BUILD_CACHE = {}


def _get_kernel(nb, na, nd):
    key = (nb, na, nd)
    if key not in _BUILD_CACHE:
        _BUILD_CACHE[key] = build_kernel(nb, na, nd)
    return _BUILD_CACHE[key]


def _shard_pad(arr, n_pad_per_core, fill=0.0):
    """[N,...] -> list of 8 per-core arrays padded to n_pad_per_core."""
    n = arr.shape[0]
    per = n // N_CORES
    out = []
    for c in range(N_CORES):
        a = arr[c * per:(c + 1) * per]
        npad = n_pad_per_core - per
        if npad:
            pad = np.full((npad,) + a.shape[1:], fill, dtype=a.dtype)
            a = np.concatenate([a, pad])
        out.append(a)
    return out


def _planes16(vecs, cols, ncomp):
    """[n_pad, ncomp] fp32 -> [P, ncomp*cols] fp16 planar."""
    v = vecs.reshape(P, cols, ncomp).transpose(0, 2, 1)  # [P, ncomp, cols]
    return np.ascontiguousarray(v.reshape(P, ncomp * cols).astype(np.float16))


def kernel(pos, bond_idcs, bond_eq_val, bond_tolerance,
           angle_idcs, angle_eq_val, angle_tolerance,
           dih_idcs, dih_eq_val):
    pos = np.asarray(pos, dtype=np.float32)
    bond_idcs = np.asarray(bond_idcs)
    angle_idcs = np.asarray(angle_idcs)
    dih_idcs = np.asarray(dih_idcs)

    nb = _cols(N_BONDS // N_CORES)
    na = _cols(N_ANGLES // N_CORES)
    nd = _cols(N_DIH // N_CORES)
    nbp, nap, ndp = nb * P, na * P, nd * P

    # ---- bonds: D = p0 - p1 (scaled) ----
    bD = (pos[bond_idcs[:, 0]] - pos[bond_idcs[:, 1]]) * SB
    b_eq = np.asarray(bond_eq_val, np.float32)
    b_t2 = np.asarray(bond_tolerance, np.float32) ** 2
    bDs = _shard_pad(bD, nbp)
    beqs = _shard_pad(b_eq, nbp)
    bt2s = _shard_pad(b_t2, nbp, fill=PAD_TOL2)

    # ---- angles: B0 = p0 - p1, B1 = p2 - p1 (scaled) ----
    aP1 = pos[angle_idcs[:, 1]]
    aB0 = (pos[angle_idcs[:, 0]] - aP1) * SA
    aB1 = (pos[angle_idcs[:, 2]] - aP1) * SA
    del aP1
    aV = np.concatenate([aB0, aB1], axis=1)  # [N,6]
    del aB0, aB1
    a_eq = np.asarray(angle_eq_val, np.float32)
    a_t2 = np.asarray(angle_tolerance, np.float32) ** 2
    aVs = _shard_pad(aV, nap)
    del aV
    aeqs = _shard_pad(a_eq, nap)
    at2s = _shard_pad(a_t2, nap, fill=PAD_TOL2)

    # ---- dihedrals: B0 = p0 - p1, U = p2 - p1, B2 = p3 - p2 (scaled) ----
    dP1 = pos[dih_idcs[:, 1]]
    dP2 = pos[dih_idcs[:, 2]]
    dB0 = (pos[dih_idcs[:, 0]] - dP1) * SD
    dU = (dP2 - dP1) * SD
    dB2 = (pos[dih_idcs[:, 3]] - dP2) * SD
    del dP1, dP2
    dV = np.concatenate([dB0, dU, dB2], axis=1)  # [N,9]
    del dB0, dU, dB2
    d_eq = np.asarray(dih_eq_val, np.float32)
    dVs = _shard_pad(dV, ndp)
    del dV
    deqs = _shard_pad(d_eq, ndp)

    nc = _get_kernel(nb, na, nd)

    in_maps = []
    for c in range(N_CORES):
        in_maps.append({
            "b_v": _planes16(bDs[c], nb, 3),
            "b_eq": beqs[c].reshape(P, nb).astype(np.float16),
            "b_t2": bt2s[c].reshape(P, nb).astype(np.float16),
            "a_v": _planes16(aVs[c], na, 6),
            "a_eq": aeqs[c].reshape(P, na).astype(np.float16),
            "a_t2": at2s[c].reshape(P, na).astype(np.float16),
            "d_v": _planes16(dVs[c], nd, 9),
            "d_eq": deqs[c].reshape(P, nd).astype(np.float16),
        })

    results = _run_spmd(nc, in_maps)

    bond_sum = 0.0
    angle_sum = 0.0
    cos_sum = 0.0
    for c in range(N_CORES):
        p = results[c]["partials"].astype(np.float64)
        bond_sum += p[:, 0].sum()
        angle_sum += p[:, 1].sum() + p[:, 2].sum()
        cos_sum += p[:, 3].sum() + p[:, 4].sum()

    # padding terms contribute exactly 0 to all three sums
    bond_energy = 1000.0 * bond_sum / N_BONDS
    angle_energy = 150.0 * angle_sum / N_ANGLES
    dih_energy = (2.0 * N_DIH - 2.0 * cos_sum) / N_DIH
    total = bond_energy + angle_energy + dih_energy
    return (np.float32(total), np.float32(bond_energy),
            np.float32(angle_energy), np.float32(dih_energy))
